# revision 11
# baseline (speedup 1.0000x reference)
"""3x3 median filter (zero-padded) on TRN2, 8 NeuronCores, bf16 datapath.

Input  x: (32, 3, 512, 512) float32
Output  : (32, 3, 512, 512) float32.

Accuracy: the median network only ever SELECTS one of its 9 inputs (min/max
ops create no new values), so the device-side bf16 result equals the bf16
rounding of the element that is the median of the rounded window. Order
statistics are 1-Lipschitz under sup-norm perturbation, so end-to-end error
is <= 2^-8 relative -- far inside the 2e-2 gate. Measured 3.4e-3.

Strategy
--------
Pure data parallel: batch dim sharded 4-per-core across 8 cores; per core
12 images (4 batch x 3 chan) in 2 groups of 6 images x 2 vertical halves.

bf16 doubles DVE tensor_tensor throughput (2x_1P perf mode) but ONLY for
unit-stride 4-byte-aligned access patterns, so the horizontal stage is
restructured from the fp32 baseline's stride-2 parity tricks into dense
shifted-field form:

  stage 1 (vertical, 5 TT/elem, all dense): row-pair (qmn,qmx) shared by
  both output-row parities, flat over the whole grid.

  stage 2 (horizontal, 12 TT/elem, all dense + aligned): per field
  F in {MN,MD,MX} build s1F[j]=F[j+1] (the ONLY odd-element shift, done
  as a ScalarE copy -- ACT is otherwise idle), then
     P[j]  = op(F[j], s1F[j])          # aligned TT, 2x
     R[j]  = op(P[j], F[j+2])          # +2 elems = 4B-aligned shift, 2x
  R[j] = sliding-3 result centered at col j+1; the final med3(Rmn,Rmd,Rmx)
  writes into an OUT grid whose per-image segment holds col c at position
  c+1, so the wide write starts at even offset 2 and the DMA store (which
  doesn't care about alignment) un-shifts.

  Output cols 0 and 511 (windows containing the zero pad column): ScalarE
  gathers P/Q values at grid positions {0,510} of both parities into one
  dense tile; 6 DVE ops of width 4*nimg + 2 per-parity writes into the
  OUT grids finish them (instead of 14 strided tiny ops).

Grid: per-image segment width 514 (even -> every segment start keeps 4B
parity). Segment positions 512..513 are scratch: stage-1 ops run flat over
the whole grid and compute garbage there; no stored output reads a garbage
lane (out cols 0/511 come from the boundary path).

Image rows 0 and 511 (windows contain the zero pad row): one small
24-partition pass. Its loads are issued up front (tiny); its compute is
issued LAST so it fills the DVE-idle tail while the final block's output
stores drain.

Stage-2 temp tiles alias aggressively (MN1<-Pmn, MX1<-Qmn, TF<-Qmx,
tmd<-Pmx, stage-1 t_o/t_e <- Rmn/Rmd buffers): DVE issue order makes every
WAR safe, and it buys the SBUF headroom for 6-image groups.

Engine budget per core (theory): DVE 17 TT/elem at 2x ~= 220us busy, ACT
~80us, DMA ~19MB. Loads on the SP+ACT HWDGE queues, stores on the GpSimd
SWDGE queue.
"""
import sys

if "/opt/trn_rl_repo" not in sys.path:
    sys.path.insert(0, "/opt/trn_rl_repo")

import numpy as np
import ml_dtypes
import concourse.bacc as bacc
import concourse.mybir as mybir
import concourse.tile as tile
from concourse import bass_utils

B, C, H, W = 32, 3, 512, 512
N_CORES = 8
B_PER = B // N_CORES          # 4 batches per core
NIMG = B_PER * C              # 12 images per core
GIMG = 6                      # images per tile group
PW = W + 2                    # per-image grid width (514, even)
FP = GIMG * PW                # flat grid width of row tiles (3084)
HH = H // 2                   # 256 rows per vertical half
P = 128                       # partitions = row pairs per half
NE = 2 * NIMG                 # partitions in the edge-rows pass (24)

BF16 = mybir.dt.bfloat16
MIN = mybir.AluOpType.min
MAX = mybir.AluOpType.max

_PROGRAM = None


def _seg(T, npart, nimg):
    """[npart, nimg, 514] per-image-segment view."""
    return T[:].rearrange("p (i w) -> p i w", w=PW)[0:npart, 0:nimg]


def _stage2_copies(nc, pm, MN, MD, MX, npart, nimg, pref):
    """ScalarE: the one odd-element shift per field, s1F[j] = F[j+1]."""
    NF = nimg * PW
    s1 = {}
    for name, F in (("MN", MN), ("MD", MD), ("MX", MX)):
        T = pm.tile([P, FP], BF16, tag=f"s1{name}", name=f"{pref}s1{name}")
        nc.scalar.copy(T[:][0:npart, 0 : NF - 1], F[:][0:npart, 1:NF])
        s1[name] = T
    return s1


def _stage2_compute(nc, pm, MN, MD, MX, s1, OUT, npart, nimg, pref, bnd=None):
    """DVE: dense aligned sliding-3 per field + final med3 -> OUT grid.
    OUT per-image position c+1 holds output col c (cols 1..510 here).
    If bnd is given (main blocks), ScalarE gathers the P/Q boundary
    columns into bnd[parity] for the deferred batched boundary pass;
    otherwise (edge pass) boundary cols are done inline."""
    NF = nimg * PW
    NI = NF - 2
    fl = lambda T, a, b: T[:][0:npart, a:b]

    def t2(tag):
        return pm.tile([P, FP], BF16, tag=tag, name=f"{pref}{tag}")

    Pmn, Pmx, Qmn, Qmx = t2("Pmn"), t2("Pmx"), t2("Qmn"), t2("Qmx")
    Rmn, Rmd, Rmx = t2("Rmn"), t2("Rmd"), t2("Rmx")
    # aliases -- disjoint lifetimes given the op order below
    tmd = pm.tile([P, FP], BF16, tag="Pmx", name=f"{pref}tmd")
    MN1 = pm.tile([P, FP], BF16, tag="Pmn", name=f"{pref}MN1")
    MX1 = pm.tile([P, FP], BF16, tag="Qmn", name=f"{pref}MX1")
    TF = pm.tile([P, FP], BF16, tag="Qmx", name=f"{pref}TF")

    tt = nc.vector.tensor_tensor
    # adjacent-column pairs (j, j+1)
    tt(fl(Pmn, 0, NI), fl(MN, 0, NI), fl(s1["MN"], 0, NI), op=MAX)
    tt(fl(Qmn, 0, NI), fl(MD, 0, NI), fl(s1["MD"], 0, NI), op=MIN)
    tt(fl(Qmx, 0, NI), fl(MD, 0, NI), fl(s1["MD"], 0, NI), op=MAX)
    tt(fl(Pmx, 0, NI), fl(MX, 0, NI), fl(s1["MX"], 0, NI), op=MIN)

    gv = lambda T: _seg(T, npart, nimg)[:, :, 0:511:510]
    if bnd is not None:
        # ScalarE pulls the {0,510} boundary columns out now so the P/Q
        # buffers can be reused (aliases above) and the boundary math can
        # run batched across both parities later
        for nm, T in (("Pmn", Pmn), ("Pmx", Pmx), ("Qmn", Qmn), ("Qmx", Qmx)):
            nc.scalar.copy(bnd[nm], gv(T))

    # close the window with the +2 (aligned) shift: R[j] ~ output col j+1
    tt(fl(Rmn, 0, NI), fl(Pmn, 0, NI), fl(MN, 2, NF), op=MAX)
    tt(fl(Rmx, 0, NI), fl(Pmx, 0, NI), fl(MX, 2, NF), op=MIN)
    tt(fl(tmd, 0, NI), fl(Qmx, 0, NI), fl(MD, 2, NF), op=MIN)
    tt(fl(Rmd, 0, NI), fl(Qmn, 0, NI), fl(tmd, 0, NI), op=MAX)
    # med3(Rmn, Rmd, Rmx); last op writes output cols 1..510 at grid
    # positions 2..511 (even start -> stays in 2x mode)
    tt(fl(MN1, 0, NI), fl(Rmn, 0, NI), fl(Rmd, 0, NI), op=MIN)
    tt(fl(MX1, 0, NI), fl(Rmn, 0, NI), fl(Rmd, 0, NI), op=MAX)
    tt(fl(TF, 0, NI), fl(MX1, 0, NI), fl(Rmx, 0, NI), op=MIN)
    ov = _seg(OUT, npart, nimg)[:, :, 2:512]
    tt(ov, _seg(MN1, npart, nimg)[:, :, 0:510],
       _seg(TF, npart, nimg)[:, :, 0:510], op=MAX)

    if bnd is None:
        # inline boundary (last block): A=max(P,0), C=min(Pmx,0),
        # B=max(Qmn,min(Qmx,0)), med3 -> OUT positions {1, 512}
        bt = lambda tag: pm.tile([P, 2 * GIMG], BF16, tag=f"i{tag}b",
                                 name=f"{pref}i{tag}b")
        bv = lambda T: T[:].rearrange("p (i c) -> p i c", c=2)[0:npart, 0:nimg]
        BA, BB, BC = bt("BA"), bt("BB"), bt("BC")
        B1, B2, B3 = bt("B1"), bt("B2"), bt("B3")
        nc.vector.tensor_scalar_max(bv(BA), gv(Pmn), 0.0)
        nc.vector.tensor_scalar_min(bv(BC), gv(Pmx), 0.0)
        nc.vector.scalar_tensor_tensor(bv(BB), gv(Qmx), 0.0, gv(Qmn),
                                       op0=MIN, op1=MAX)
        tt(bv(B1), bv(BA), bv(BB), op=MIN)
        tt(bv(B2), bv(BA), bv(BB), op=MAX)
        tt(bv(B3), bv(B2), bv(BC), op=MIN)
        obv = _seg(OUT, npart, nimg)[:, :, 1:513:511]
        tt(obv, bv(B1), bv(B3), op=MAX)


def _boundary_batch(nc, pm, BP, OUT_o, OUT_e):
    """Output cols 0 and 511 for both parities in one dense pass.
    BP[nm] tiles are [P, 2*GIMG*2] with layout (parity, img, col)."""
    tt = nc.vector.tensor_tensor
    bt = lambda tag: pm.tile([P, 4 * GIMG], BF16, tag=f"{tag}b", name=f"{tag}b")
    BA, BB, BC = bt("BA"), bt("BB"), bt("BC")
    B1, B2, B3 = bt("B1"), bt("B2"), bt("B3")
    nc.vector.tensor_scalar_max(BA[:], BP["Pmn"][:], 0.0)
    nc.vector.tensor_scalar_min(BC[:], BP["Pmx"][:], 0.0)
    nc.vector.scalar_tensor_tensor(BB[:], BP["Qmx"][:], 0.0, BP["Qmn"][:],
                                   op0=MIN, op1=MAX)
    tt(B1[:], BA[:], BB[:], op=MIN)
    tt(B2[:], BA[:], BB[:], op=MAX)
    tt(B3[:], B2[:], BC[:], op=MIN)
    pv = lambda T, h: T[:].rearrange("p (h i c) -> p h i c", h=2, c=2)[:, h]
    for h, OUT in ((0, OUT_o), (1, OUT_e)):
        obv = _seg(OUT, P, GIMG)[:, :, 1:513:511]
        tt(obv, pv(B1, h), pv(B3, h), op=MAX)


def _block(nc, pio, pm, xh, oh, g, half, last=False):
    """One vertical half of one image group: odd output rows r0+1..r0+255,
    even rows r0+2..r0+256 (halves overlap by two rows so every DMA is a
    full 128-partition transfer). Rows 0 and 511 via the edge pass.
    last=True: per-parity boundary + store OUT_o right after parity o and
    OUT_e on the (by then idle) sync HWDGE queue, shrinking the final
    store-drain tail."""
    r0 = 0 if half == 0 else H - HH - 2
    i0 = GIMG * g

    E = pio.tile([P, FP], BF16, tag="E", name="E", bufs=2)
    O = pio.tile([P, FP], BF16, tag="O", name="O", bufs=2)
    E_sh = pio.tile([P, FP], BF16, tag="E_sh", name="E_sh", bufs=2)
    O_sh2 = pio.tile([P, FP], BF16, tag="O_sh2", name="O_sh2", bufs=2)

    # scratch cols 512..513 of each segment are read by the flat stage-1
    # ops but never loaded: define them so no lane is uninitialized
    for T in (E, O, E_sh, O_sh2):
        nc.gpsimd.memset(_seg(T, P, GIMG)[:, :, W:PW], 0.0)

    lv = lambda T: _seg(T, P, GIMG)[:, :, 0:W]
    img = lambda r_lo: xh[r_lo : min(r_lo + 2 * P, H) : 2, i0 : i0 + GIMG, :]
    # queue order matters (HWDGE queues are FIFOs): the (O, E_sh) pair
    # feeds the first op of the block, so those loads go first
    nc.sync.dma_start(lv(E_sh), img(r0 + 2))     # rows r0+2p+2
    nc.scalar.dma_start(lv(O), img(r0 + 1))      # rows r0+2p+1
    nc.sync.dma_start(lv(E), img(r0))            # rows r0+2p
    nc.scalar.dma_start(lv(O_sh2), img(r0 + 3))  # rows r0+2p+3

    # stage 1: shared pair = (O, E_sh) = rows (2p+1, 2p+2); flat ops
    qmn = pm.tile([P, FP], BF16, tag="qmn", name="qmn")
    qmx = pm.tile([P, FP], BF16, tag="qmx", name="qmx")
    tt = nc.vector.tensor_tensor
    tt(qmn[:], O[:], E_sh[:], op=MIN)
    tt(qmx[:], O[:], E_sh[:], op=MAX)

    def fld(tag):
        return pm.tile([P, FP], BF16, tag=tag, name=tag)

    MN_o, MD_o, MX_o = fld("MN_o"), fld("MD_o"), fld("MX_o")
    MN_e, MD_e, MX_e = fld("MN_e"), fld("MD_e"), fld("MX_e")
    # stage-1 temps alias stage-2 R slots (dead before those are written)
    t_o = pm.tile([P, FP], BF16, tag="Rmn", name="t_o")
    t_e = pm.tile([P, FP], BF16, tag="Rmd", name="t_e")

    # odd output rows r0+2p+1: pair + E (row r0+2p)
    tt(MN_o[:], qmn[:], E[:], op=MIN)
    tt(MX_o[:], qmx[:], E[:], op=MAX)
    tt(t_o[:], qmx[:], E[:], op=MIN)
    tt(MD_o[:], qmn[:], t_o[:], op=MAX)
    # even output rows r0+2p+2: pair + O_sh2 (row r0+2p+3)
    tt(MN_e[:], qmn[:], O_sh2[:], op=MIN)
    tt(MX_e[:], qmx[:], O_sh2[:], op=MAX)
    tt(t_e[:], qmx[:], O_sh2[:], op=MIN)
    tt(MD_e[:], qmn[:], t_e[:], op=MAX)

    # boundary gather tiles: layout (parity, img, col{0,510})
    BP = {
        nm: pm.tile([P, 4 * GIMG], BF16, tag=f"BP{nm}", name=f"BP{nm}")
        for nm in ("Pmn", "Pmx", "Qmn", "Qmx")
    }
    hv = lambda nm, h: BP[nm][:].rearrange("p (h i c) -> p h i c", h=2, c=2)[:, h]

    OUT_o = pio.tile([P, FP], BF16, tag="OUT_o", name="OUT_o")
    OUT_e = pio.tile([P, FP], BF16, tag="OUT_e", name="OUT_e")
    out_img = lambda r_lo: oh[r_lo : min(r_lo + 2 * P, H) : 2, i0 : i0 + GIMG, :]
    ostore = lambda T: _seg(T, P, GIMG)[:, :, 1:513]

    s1_o = _stage2_copies(nc, pm, MN_o, MD_o, MX_o, P, GIMG, "o_")
    _stage2_compute(nc, pm, MN_o, MD_o, MX_o, s1_o, OUT_o, P, GIMG, "o_",
                    bnd=None if last else {nm: hv(nm, 0) for nm in BP})
    if last:
        nc.gpsimd.dma_start(out_img(r0 + 1), ostore(OUT_o))
    s1_e = _stage2_copies(nc, pm, MN_e, MD_e, MX_e, P, GIMG, "e_")
    _stage2_compute(nc, pm, MN_e, MD_e, MX_e, s1_e, OUT_e, P, GIMG, "e_",
                    bnd=None if last else {nm: hv(nm, 1) for nm in BP})
    if last:
        nc.sync.dma_start(out_img(r0 + 2), ostore(OUT_e))
        return
    _boundary_batch(nc, pm, BP, OUT_o, OUT_e)

    # stores on the SWDGE queue so they never block a later block's loads
    nc.gpsimd.dma_start(out_img(r0 + 1), ostore(OUT_o))
    nc.gpsimd.dma_start(out_img(r0 + 2), ostore(OUT_e))


def _edge_loads(nc, pio, xi):
    """Loads for image rows 0 and 511 (tiny, partial-partition): issued up
    front so the end-of-kernel edge compute never waits on DMA."""
    R0 = pio.tile([NE, PW], BF16, tag="R0", name="R0")
    R1 = pio.tile([NE, PW], BF16, tag="R1", name="R1")
    for T in (R0, R1):
        nc.gpsimd.memset(T[:][0:NE, W:PW], 0.0)
    nc.sync.dma_start(R0[:][0:NIMG, 0:W], xi[:, 0, :])
    nc.scalar.dma_start(R1[:][0:NIMG, 0:W], xi[:, 1, :])
    nc.sync.dma_start(R0[:][NIMG:NE, 0:W], xi[:, H - 1, :])
    nc.scalar.dma_start(R1[:][NIMG:NE, 0:W], xi[:, H - 2, :])
    return R0, R1


def _edge_compute(nc, pio, pm, oi, R0, R1):
    """Rows 0 and 511 (windows contain the zero pad row), 24 partitions:
    p 0..11 = row 0 of image p; p 12..23 = row 511 of image p-12. Runs
    last, in the shadow of the final block's output stores."""
    rmn = pm.tile([NE, PW], BF16, tag="rmn", name="rmn")
    rmx = pm.tile([NE, PW], BF16, tag="rmx", name="rmx")
    nc.vector.tensor_tensor(rmn[:], R0[:], R1[:], op=MIN)
    nc.vector.tensor_tensor(rmx[:], R0[:], R1[:], op=MAX)

    # vertical sort3 with the zero pad row: min/max vs 0, med=max(mn,min(mx,0))
    MN0 = pm.tile([NE, PW], BF16, tag="eMN", name="eMN")
    MD0 = pm.tile([NE, PW], BF16, tag="eMD", name="eMD")
    MX0 = pm.tile([NE, PW], BF16, tag="eMX", name="eMX")
    nc.vector.tensor_scalar_min(MN0[:], rmn[:], 0.0)
    nc.vector.tensor_scalar_max(MX0[:], rmx[:], 0.0)
    nc.vector.scalar_tensor_tensor(MD0[:], rmx[:], 0.0, rmn[:], op0=MIN, op1=MAX)

    s1 = {}
    for name, F in (("MN", MN0), ("MD", MD0), ("MX", MX0)):
        T = pm.tile([NE, PW], BF16, tag=f"es1{name}", name=f"es1{name}")
        nc.scalar.copy(T[:][0:NE, 0 : PW - 1], F[:][0:NE, 1:PW])
        s1[name] = T

    OUT0 = pio.tile([NE, PW], BF16, tag="OUT0", name="OUT0")
    _stage2_compute_small(nc, pm, MN0, MD0, MX0, s1, OUT0)
    nc.gpsimd.dma_start(oi[:, 0, :], OUT0[:][0:NIMG, 1:513])
    nc.gpsimd.dma_start(oi[:, H - 1, :], OUT0[:][NIMG:NE, 1:513])


def _stage2_compute_small(nc, pm, MN, MD, MX, s1, OUT):
    """Same dataflow as _stage2_compute on [NE, PW] tiles (nimg=1)."""
    NF = PW
    NI = NF - 2
    fl = lambda T, a, b: T[:][0:NE, a:b]

    def t2(tag):
        return pm.tile([NE, PW], BF16, tag=f"e{tag}", name=f"e{tag}")

    Pmn, Pmx, Qmn, Qmx = t2("Pmn"), t2("Pmx"), t2("Qmn"), t2("Qmx")
    tmd, Rmn, Rmd, Rmx = t2("tmd"), t2("Rmn"), t2("Rmd"), t2("Rmx")
    MN1, MX1, TF = t2("MN1"), t2("MX1"), t2("TF")

    tt = nc.vector.tensor_tensor
    tt(fl(Pmn, 0, NI), fl(MN, 0, NI), fl(s1["MN"], 0, NI), op=MAX)
    tt(fl(Qmn, 0, NI), fl(MD, 0, NI), fl(s1["MD"], 0, NI), op=MIN)
    tt(fl(Qmx, 0, NI), fl(MD, 0, NI), fl(s1["MD"], 0, NI), op=MAX)
    tt(fl(Pmx, 0, NI), fl(MX, 0, NI), fl(s1["MX"], 0, NI), op=MIN)
    tt(fl(Rmn, 0, NI), fl(Pmn, 0, NI), fl(MN, 2, NF), op=MAX)
    tt(fl(tmd, 0, NI), fl(Qmx, 0, NI), fl(MD, 2, NF), op=MIN)
    tt(fl(Rmd, 0, NI), fl(Qmn, 0, NI), fl(tmd, 0, NI), op=MAX)
    tt(fl(Rmx, 0, NI), fl(Pmx, 0, NI), fl(MX, 2, NF), op=MIN)
    tt(fl(MN1, 0, NI), fl(Rmn, 0, NI), fl(Rmd, 0, NI), op=MIN)
    tt(fl(MX1, 0, NI), fl(Rmn, 0, NI), fl(Rmd, 0, NI), op=MAX)
    tt(fl(TF, 0, NI), fl(MX1, 0, NI), fl(Rmx, 0, NI), op=MIN)
    tt(fl(OUT, 2, 512), fl(MN1, 0, 510), fl(TF, 0, 510), op=MAX)

    gv = lambda T: T[:][0:NE, 0:511:510]
    bt = lambda tag: pm.tile([NE, 2], BF16, tag=f"e{tag}b", name=f"e{tag}b")
    BA, BB, BC = bt("BA"), bt("BB"), bt("BC")
    B1, B2, B3 = bt("B1"), bt("B2"), bt("B3")
    nc.vector.tensor_scalar_max(BA[:], gv(Pmn), 0.0)
    nc.vector.tensor_scalar_min(BC[:], gv(Pmx), 0.0)
    nc.vector.scalar_tensor_tensor(BB[:], gv(Qmx), 0.0, gv(Qmn), op0=MIN, op1=MAX)
    tt(B1[:], BA[:], BB[:], op=MIN)
    tt(B2[:], BA[:], BB[:], op=MAX)
    tt(B3[:], B2[:], BC[:], op=MIN)
    tt(OUT[:][0:NE, 1:513:511], B1[:], B3[:], op=MAX)


def build_program():
    nc = bacc.Bacc(
        "TRN2", target_bir_lowering=False, debug=False, num_devices=N_CORES
    )
    x_d = nc.dram_tensor("x", [B_PER, C, H, W], BF16, kind="ExternalInput").ap()
    o_d = nc.dram_tensor("out", [B_PER, C, H, W], BF16, kind="ExternalOutput").ap()
    xh = x_d.rearrange("b c h w -> h (b c) w")  # [512, 12, 512]
    oh = o_d.rearrange("b c h w -> h (b c) w")
    xi = x_d.rearrange("b c h w -> (b c) h w")  # [12, 512, 512]
    oi = o_d.rearrange("b c h w -> (b c) h w")

    with tile.TileContext(nc) as tc:
        with (
            tc.tile_pool(name="io", bufs=1) as pio,
            tc.tile_pool(name="mid", bufs=1) as pm,
        ):
            # edge pass first: its tiny loads land fast, and its 6us of DVE
            # work fills the window while block 0's big loads stream in
            R0, R1 = _edge_loads(nc, pio, xi)
            _edge_compute(nc, pio, pm, oi, R0, R1)
            nb = 2 * (NIMG // GIMG)
            for i in range(nb):
                _block(nc, pio, pm, xh, oh, i // 2, i % 2, last=(i == nb - 1))
    nc.compile()
    return nc


def _get_program():
    global _PROGRAM
    if _PROGRAM is None:
        _PROGRAM = build_program()
    return _PROGRAM


def make_in_maps(x: np.ndarray):
    xb = np.ascontiguousarray(x).astype(ml_dtypes.bfloat16)
    return [{"x": xb[k * B_PER : (k + 1) * B_PER]} for k in range(N_CORES)]


def kernel(**inputs) -> np.ndarray:
    x = np.asarray(inputs["x"], dtype=np.float32)
    assert x.shape == (B, C, H, W), x.shape
    nc = _get_program()
    res = bass_utils.run_bass_kernel_spmd(
        nc, make_in_maps(x), core_ids=list(range(N_CORES))
    )
    out = np.concatenate(
        [np.asarray(res.results[k]["out"]) for k in range(N_CORES)], axis=0
    )
    return out.astype(np.float32)


# revision 14
# speedup vs baseline: 1.0209x; 1.0209x over previous
"""3x3 median filter (zero-padded) on TRN2, 8 NeuronCores, bf16 datapath.

Input  x: (32, 3, 512, 512) float32
Output  : (32, 3, 512, 512) float32.

Accuracy: the median network only ever SELECTS one of its 9 inputs (min/max
ops create no new values), so the device-side bf16 result equals the bf16
rounding of the element that is the median of the rounded window. Order
statistics are 1-Lipschitz under sup-norm perturbation, so end-to-end error
is <= 2^-8 relative -- far inside the 2e-2 gate. Measured 3.4e-3.

Strategy
--------
Pure data parallel: batch dim sharded 4-per-core across 8 cores; per core
12 images (4 batch x 3 chan) in 2 groups of 6 images x 2 vertical halves.

bf16 doubles DVE tensor_tensor throughput (2x_1P perf mode) but ONLY for
unit-stride 4-byte-aligned access patterns, so the horizontal stage is
restructured from the fp32 baseline's stride-2 parity tricks into dense
shifted-field form:

  stage 1 (vertical, 5 TT/elem, all dense): row-pair (qmn,qmx) shared by
  both output-row parities, flat over the whole grid.

  stage 2 (horizontal, 12 TT/elem, all dense + aligned): per field
  F in {MN,MD,MX} build s1F[j]=F[j+1] (the ONLY odd-element shift, done
  as a ScalarE copy -- ACT is otherwise idle), then
     P[j]  = op(F[j], s1F[j])          # aligned TT, 2x
     R[j]  = op(P[j], F[j+2])          # +2 elems = 4B-aligned shift, 2x
  R[j] = sliding-3 result centered at col j+1; the final med3(Rmn,Rmd,Rmx)
  writes into an OUT grid whose per-image segment holds col c at position
  c+1, so the wide write starts at even offset 2 and the DMA store (which
  doesn't care about alignment) un-shifts.

  Output cols 0 and 511 (windows containing the zero pad column): ScalarE
  gathers P/Q values at grid positions {0,510} of both parities into one
  dense tile; 6 DVE ops of width 4*nimg + 2 per-parity writes into the
  OUT grids finish them (instead of 14 strided tiny ops).

Grid: per-image segment width 514 (even -> every segment start keeps 4B
parity). Segment positions 512..513 are scratch: stage-1 ops run flat over
the whole grid and compute garbage there; no stored output reads a garbage
lane (out cols 0/511 come from the boundary path).

Image rows 0 and 511 (windows contain the zero pad row): one small
24-partition pass. Its loads are issued up front (tiny); its compute is
issued LAST so it fills the DVE-idle tail while the final block's output
stores drain.

Stage-2 temp tiles alias aggressively (MN1<-Pmn, MX1<-Qmn, TF<-Qmx,
tmd<-Pmx, stage-1 t_o/t_e <- Rmn/Rmd buffers): DVE issue order makes every
WAR safe, and it buys the SBUF headroom for 6-image groups.

Engine budget per core (theory): DVE 17 TT/elem at 2x ~= 220us busy, ACT
~80us, DMA ~19MB. Loads on the SP+ACT HWDGE queues, stores on the GpSimd
SWDGE queue.
"""
import sys

if "/opt/trn_rl_repo" not in sys.path:
    sys.path.insert(0, "/opt/trn_rl_repo")

import numpy as np
import ml_dtypes
import concourse.bacc as bacc
import concourse.mybir as mybir
import concourse.tile as tile
from concourse import bass_utils

B, C, H, W = 32, 3, 512, 512
N_CORES = 8
B_PER = B // N_CORES          # 4 batches per core
NIMG = B_PER * C              # 12 images per core
GIMG = 6                      # images per tile group
PW = W + 2                    # per-image grid width (514, even)
FP = GIMG * PW                # flat grid width of row tiles (3084)
HH = H // 2                   # 256 rows per vertical half
P = 128                       # partitions = row pairs per half
NE = 2 * NIMG                 # partitions in the edge-rows pass (24)

BF16 = mybir.dt.bfloat16
MIN = mybir.AluOpType.min
MAX = mybir.AluOpType.max

_PROGRAM = None


def _seg(T, npart, nimg):
    """[npart, nimg, 514] per-image-segment view."""
    return T[:].rearrange("p (i w) -> p i w", w=PW)[0:npart, 0:nimg]


def _stage2_copies(nc, pm, MN, MD, MX, npart, nimg, pref):
    """ScalarE: the one odd-element shift per field, s1F[j] = F[j+1]."""
    NF = nimg * PW
    s1 = {}
    for name, F in (("MN", MN), ("MD", MD), ("MX", MX)):
        T = pm.tile([P, FP], BF16, tag=f"s1{name}", name=f"{pref}s1{name}")
        nc.scalar.copy(T[:][0:npart, 0 : NF - 1], F[:][0:npart, 1:NF])
        s1[name] = T
    return s1


def _stage2_compute(nc, pm, MN, MD, MX, s1, OUT, npart, nimg, pref, bnd=None):
    """DVE: dense aligned sliding-3 per field + final med3 -> OUT grid.
    OUT per-image position c+1 holds output col c (cols 1..510 here).
    If bnd is given (main blocks), ScalarE gathers the P/Q boundary
    columns into bnd[parity] for the deferred batched boundary pass;
    otherwise (edge pass) boundary cols are done inline."""
    NF = nimg * PW
    NI = NF - 2
    fl = lambda T, a, b: T[:][0:npart, a:b]

    def t2(tag):
        return pm.tile([P, FP], BF16, tag=tag, name=f"{pref}{tag}")

    Pmn, Pmx, Qmn, Qmx = t2("Pmn"), t2("Pmx"), t2("Qmn"), t2("Qmx")
    Rmn, Rmd, Rmx = t2("Rmn"), t2("Rmd"), t2("Rmx")
    # aliases -- disjoint lifetimes given the op order below
    tmd = pm.tile([P, FP], BF16, tag="Pmx", name=f"{pref}tmd")
    MN1 = pm.tile([P, FP], BF16, tag="Pmn", name=f"{pref}MN1")
    MX1 = pm.tile([P, FP], BF16, tag="Qmn", name=f"{pref}MX1")
    TF = pm.tile([P, FP], BF16, tag="Qmx", name=f"{pref}TF")

    tt = nc.vector.tensor_tensor
    # adjacent-column pairs (j, j+1)
    tt(fl(Pmn, 0, NI), fl(MN, 0, NI), fl(s1["MN"], 0, NI), op=MAX)
    tt(fl(Qmn, 0, NI), fl(MD, 0, NI), fl(s1["MD"], 0, NI), op=MIN)
    tt(fl(Qmx, 0, NI), fl(MD, 0, NI), fl(s1["MD"], 0, NI), op=MAX)
    tt(fl(Pmx, 0, NI), fl(MX, 0, NI), fl(s1["MX"], 0, NI), op=MIN)

    gv = lambda T: _seg(T, npart, nimg)[:, :, 0:511:510]
    if bnd is not None:
        # ScalarE pulls the {0,510} boundary columns out now so the P/Q
        # buffers can be reused (aliases above) and the boundary math can
        # run batched across both parities later
        for nm, T in (("Pmn", Pmn), ("Pmx", Pmx), ("Qmn", Qmn), ("Qmx", Qmx)):
            nc.scalar.copy(bnd[nm], gv(T))

    # close the window with the +2 (aligned) shift: R[j] ~ output col j+1
    tt(fl(Rmn, 0, NI), fl(Pmn, 0, NI), fl(MN, 2, NF), op=MAX)
    tt(fl(Rmx, 0, NI), fl(Pmx, 0, NI), fl(MX, 2, NF), op=MIN)
    tt(fl(tmd, 0, NI), fl(Qmx, 0, NI), fl(MD, 2, NF), op=MIN)
    tt(fl(Rmd, 0, NI), fl(Qmn, 0, NI), fl(tmd, 0, NI), op=MAX)
    # med3(Rmn, Rmd, Rmx); last op writes output cols 1..510 at grid
    # positions 2..511 (even start -> stays in 2x mode)
    tt(fl(MN1, 0, NI), fl(Rmn, 0, NI), fl(Rmd, 0, NI), op=MIN)
    tt(fl(MX1, 0, NI), fl(Rmn, 0, NI), fl(Rmd, 0, NI), op=MAX)
    tt(fl(TF, 0, NI), fl(MX1, 0, NI), fl(Rmx, 0, NI), op=MIN)
    ov = _seg(OUT, npart, nimg)[:, :, 2:512]
    tt(ov, _seg(MN1, npart, nimg)[:, :, 0:510],
       _seg(TF, npart, nimg)[:, :, 0:510], op=MAX)

    if bnd is None:
        # inline boundary (last block): A=max(P,0), C=min(Pmx,0),
        # B=max(Qmn,min(Qmx,0)), med3 -> OUT positions {1, 512}
        bt = lambda tag: pm.tile([P, 2 * GIMG], BF16, tag=f"i{tag}b",
                                 name=f"{pref}i{tag}b")
        bv = lambda T: T[:].rearrange("p (i c) -> p i c", c=2)[0:npart, 0:nimg]
        BA, BB, BC = bt("BA"), bt("BB"), bt("BC")
        B1, B2, B3 = bt("B1"), bt("B2"), bt("B3")
        nc.vector.tensor_scalar_max(bv(BA), gv(Pmn), 0.0)
        nc.vector.tensor_scalar_min(bv(BC), gv(Pmx), 0.0)
        nc.vector.scalar_tensor_tensor(bv(BB), gv(Qmx), 0.0, gv(Qmn),
                                       op0=MIN, op1=MAX)
        tt(bv(B1), bv(BA), bv(BB), op=MIN)
        tt(bv(B2), bv(BA), bv(BB), op=MAX)
        tt(bv(B3), bv(B2), bv(BC), op=MIN)
        obv = _seg(OUT, npart, nimg)[:, :, 1:513:511]
        tt(obv, bv(B1), bv(B3), op=MAX)


def _boundary_batch(nc, pm, BP, OUT_o, OUT_e):
    """Output cols 0 and 511 for both parities in one dense pass.
    BP[nm] tiles are [P, 2*GIMG*2] with layout (parity, img, col)."""
    tt = nc.vector.tensor_tensor
    bt = lambda tag: pm.tile([P, 4 * GIMG], BF16, tag=f"{tag}b", name=f"{tag}b")
    BA, BB, BC = bt("BA"), bt("BB"), bt("BC")
    B1, B2, B3 = bt("B1"), bt("B2"), bt("B3")
    nc.vector.tensor_scalar_max(BA[:], BP["Pmn"][:], 0.0)
    nc.vector.tensor_scalar_min(BC[:], BP["Pmx"][:], 0.0)
    nc.vector.scalar_tensor_tensor(BB[:], BP["Qmx"][:], 0.0, BP["Qmn"][:],
                                   op0=MIN, op1=MAX)
    tt(B1[:], BA[:], BB[:], op=MIN)
    tt(B2[:], BA[:], BB[:], op=MAX)
    tt(B3[:], B2[:], BC[:], op=MIN)
    pv = lambda T, h: T[:].rearrange("p (h i c) -> p h i c", h=2, c=2)[:, h]
    for h, OUT in ((0, OUT_o), (1, OUT_e)):
        obv = _seg(OUT, P, GIMG)[:, :, 1:513:511]
        tt(obv, pv(B1, h), pv(B3, h), op=MAX)


def _block(nc, pio, pm, xh, oh, g, half, first=False, last=False):
    """One vertical half of one image group: odd output rows r0+1..r0+255,
    even rows r0+2..r0+256 (halves overlap by two rows so every DMA is a
    full 128-partition transfer). Rows 0 and 511 via the edge pass.
    first=True: loads and stage 1 split into two 3-image chunks so the DVE
    starts after ~1.5MB has landed instead of ~3MB (cold-start only).
    last=True: the two output stores go to the by-then-idle HWDGE queues
    so the final drain overlaps the edge pass."""
    r0 = 0 if half == 0 else H - HH - 2
    i0 = GIMG * g

    E = pio.tile([P, FP], BF16, tag="E", name="E", bufs=2)
    O = pio.tile([P, FP], BF16, tag="O", name="O", bufs=2)
    E_sh = pio.tile([P, FP], BF16, tag="E_sh", name="E_sh", bufs=2)
    O_sh2 = pio.tile([P, FP], BF16, tag="O_sh2", name="O_sh2", bufs=2)

    # scratch cols 512..513 of each segment are read by the flat stage-1
    # ops but never loaded: define them so no lane is uninitialized
    for T in (E, O, E_sh, O_sh2):
        nc.gpsimd.memset(_seg(T, P, GIMG)[:, :, W:PW], 0.0)

    def loads(ia, ib):
        lv = lambda T: _seg(T, P, GIMG)[:, ia:ib, 0:W]
        im = lambda r_lo: xh[
            r_lo : min(r_lo + 2 * P, H) : 2, i0 + ia : i0 + ib, :
        ]
        # queue order matters (HWDGE queues are FIFOs): the (O, E_sh) pair
        # feeds the first op of the block, so those loads go first
        nc.sync.dma_start(lv(E_sh), im(r0 + 2))     # rows r0+2p+2
        nc.scalar.dma_start(lv(O), im(r0 + 1))      # rows r0+2p+1
        nc.sync.dma_start(lv(E), im(r0))            # rows r0+2p
        nc.scalar.dma_start(lv(O_sh2), im(r0 + 3))  # rows r0+2p+3

    # stage 1: shared pair = (O, E_sh) = rows (2p+1, 2p+2); flat ops
    qmn = pm.tile([P, FP], BF16, tag="qmn", name="qmn")
    qmx = pm.tile([P, FP], BF16, tag="qmx", name="qmx")
    tt = nc.vector.tensor_tensor

    def fld(tag):
        return pm.tile([P, FP], BF16, tag=tag, name=tag)

    MN_o, MD_o, MX_o = fld("MN_o"), fld("MD_o"), fld("MX_o")
    MN_e, MD_e, MX_e = fld("MN_e"), fld("MD_e"), fld("MX_e")
    # stage-1 temps alias stage-2 R slots (dead before those are written)
    t_o = pm.tile([P, FP], BF16, tag="Rmn", name="t_o")
    t_e = pm.tile([P, FP], BF16, tag="Rmd", name="t_e")

    def stage1(ia, ib):
        fv = lambda T: T[:][:, ia * PW : ib * PW]
        tt(fv(qmn), fv(O), fv(E_sh), op=MIN)
        tt(fv(qmx), fv(O), fv(E_sh), op=MAX)
        # odd output rows r0+2p+1: pair + E (row r0+2p)
        tt(fv(MN_o), fv(qmn), fv(E), op=MIN)
        tt(fv(MX_o), fv(qmx), fv(E), op=MAX)
        tt(fv(t_o), fv(qmx), fv(E), op=MIN)
        tt(fv(MD_o), fv(qmn), fv(t_o), op=MAX)
        # even output rows r0+2p+2: pair + O_sh2 (row r0+2p+3)
        tt(fv(MN_e), fv(qmn), fv(O_sh2), op=MIN)
        tt(fv(MX_e), fv(qmx), fv(O_sh2), op=MAX)
        tt(fv(t_e), fv(qmx), fv(O_sh2), op=MIN)
        tt(fv(MD_e), fv(qmn), fv(t_e), op=MAX)

    if first:
        hg = GIMG // 2
        loads(0, hg)
        loads(hg, GIMG)
        stage1(0, hg)
        stage1(hg, GIMG)
    else:
        loads(0, GIMG)
        stage1(0, GIMG)

    # boundary gather tiles: layout (parity, img, col{0,510})
    BP = {
        nm: pm.tile([P, 4 * GIMG], BF16, tag=f"BP{nm}", name=f"BP{nm}")
        for nm in ("Pmn", "Pmx", "Qmn", "Qmx")
    }
    hv = lambda nm, h: BP[nm][:].rearrange("p (h i c) -> p h i c", h=2, c=2)[:, h]

    OUT_o = pio.tile([P, FP], BF16, tag="OUT_o", name="OUT_o")
    OUT_e = pio.tile([P, FP], BF16, tag="OUT_e", name="OUT_e")
    out_img = lambda r_lo: oh[r_lo : min(r_lo + 2 * P, H) : 2, i0 : i0 + GIMG, :]
    ostore = lambda T: _seg(T, P, GIMG)[:, :, 1:513]

    s1_o = _stage2_copies(nc, pm, MN_o, MD_o, MX_o, P, GIMG, "o_")
    _stage2_compute(nc, pm, MN_o, MD_o, MX_o, s1_o, OUT_o, P, GIMG, "o_",
                    bnd={nm: hv(nm, 0) for nm in BP})
    s1_e = _stage2_copies(nc, pm, MN_e, MD_e, MX_e, P, GIMG, "e_")
    _stage2_compute(nc, pm, MN_e, MD_e, MX_e, s1_e, OUT_e, P, GIMG, "e_",
                    bnd={nm: hv(nm, 1) for nm in BP})
    _boundary_batch(nc, pm, BP, OUT_o, OUT_e)

    if last:
        # HWDGE queues are idle by now (all loads issued); their stores
        # drain while the edge pass runs, shrinking the end-of-kernel tail
        nc.sync.dma_start(out_img(r0 + 1), ostore(OUT_o))
        nc.scalar.dma_start(out_img(r0 + 2), ostore(OUT_e))
    else:
        # stores on the SWDGE queue so they never block later blocks' loads
        nc.gpsimd.dma_start(out_img(r0 + 1), ostore(OUT_o))
        nc.gpsimd.dma_start(out_img(r0 + 2), ostore(OUT_e))


def _edge_loads(nc, pio, xi):
    """Loads for image rows 0 and 511 (tiny, partial-partition): issued up
    front so the end-of-kernel edge compute never waits on DMA."""
    R0 = pio.tile([NE, PW], BF16, tag="R0", name="R0")
    R1 = pio.tile([NE, PW], BF16, tag="R1", name="R1")
    for T in (R0, R1):
        nc.gpsimd.memset(T[:][0:NE, W:PW], 0.0)
    nc.sync.dma_start(R0[:][0:NIMG, 0:W], xi[:, 0, :])
    nc.scalar.dma_start(R1[:][0:NIMG, 0:W], xi[:, 1, :])
    nc.sync.dma_start(R0[:][NIMG:NE, 0:W], xi[:, H - 1, :])
    nc.scalar.dma_start(R1[:][NIMG:NE, 0:W], xi[:, H - 2, :])
    return R0, R1


def _edge_compute(nc, pio, pm, oi, R0, R1):
    """Rows 0 and 511 (windows contain the zero pad row), 24 partitions:
    p 0..11 = row 0 of image p; p 12..23 = row 511 of image p-12. Runs
    last, in the shadow of the final block's output stores."""
    rmn = pm.tile([NE, PW], BF16, tag="rmn", name="rmn")
    rmx = pm.tile([NE, PW], BF16, tag="rmx", name="rmx")
    nc.vector.tensor_tensor(rmn[:], R0[:], R1[:], op=MIN)
    nc.vector.tensor_tensor(rmx[:], R0[:], R1[:], op=MAX)

    # vertical sort3 with the zero pad row: min/max vs 0, med=max(mn,min(mx,0))
    MN0 = pm.tile([NE, PW], BF16, tag="eMN", name="eMN")
    MD0 = pm.tile([NE, PW], BF16, tag="eMD", name="eMD")
    MX0 = pm.tile([NE, PW], BF16, tag="eMX", name="eMX")
    nc.vector.tensor_scalar_min(MN0[:], rmn[:], 0.0)
    nc.vector.tensor_scalar_max(MX0[:], rmx[:], 0.0)
    nc.vector.scalar_tensor_tensor(MD0[:], rmx[:], 0.0, rmn[:], op0=MIN, op1=MAX)

    s1 = {}
    for name, F in (("MN", MN0), ("MD", MD0), ("MX", MX0)):
        T = pm.tile([NE, PW], BF16, tag=f"es1{name}", name=f"es1{name}")
        nc.scalar.copy(T[:][0:NE, 0 : PW - 1], F[:][0:NE, 1:PW])
        s1[name] = T

    OUT0 = pio.tile([NE, PW], BF16, tag="OUT0", name="OUT0")
    _stage2_compute_small(nc, pm, MN0, MD0, MX0, s1, OUT0)
    nc.gpsimd.dma_start(oi[:, 0, :], OUT0[:][0:NIMG, 1:513])
    nc.gpsimd.dma_start(oi[:, H - 1, :], OUT0[:][NIMG:NE, 1:513])


def _stage2_compute_small(nc, pm, MN, MD, MX, s1, OUT):
    """Same dataflow as _stage2_compute on [NE, PW] tiles (nimg=1)."""
    NF = PW
    NI = NF - 2
    fl = lambda T, a, b: T[:][0:NE, a:b]

    def t2(tag):
        return pm.tile([NE, PW], BF16, tag=f"e{tag}", name=f"e{tag}")

    Pmn, Pmx, Qmn, Qmx = t2("Pmn"), t2("Pmx"), t2("Qmn"), t2("Qmx")
    tmd, Rmn, Rmd, Rmx = t2("tmd"), t2("Rmn"), t2("Rmd"), t2("Rmx")
    MN1, MX1, TF = t2("MN1"), t2("MX1"), t2("TF")

    tt = nc.vector.tensor_tensor
    tt(fl(Pmn, 0, NI), fl(MN, 0, NI), fl(s1["MN"], 0, NI), op=MAX)
    tt(fl(Qmn, 0, NI), fl(MD, 0, NI), fl(s1["MD"], 0, NI), op=MIN)
    tt(fl(Qmx, 0, NI), fl(MD, 0, NI), fl(s1["MD"], 0, NI), op=MAX)
    tt(fl(Pmx, 0, NI), fl(MX, 0, NI), fl(s1["MX"], 0, NI), op=MIN)
    tt(fl(Rmn, 0, NI), fl(Pmn, 0, NI), fl(MN, 2, NF), op=MAX)
    tt(fl(tmd, 0, NI), fl(Qmx, 0, NI), fl(MD, 2, NF), op=MIN)
    tt(fl(Rmd, 0, NI), fl(Qmn, 0, NI), fl(tmd, 0, NI), op=MAX)
    tt(fl(Rmx, 0, NI), fl(Pmx, 0, NI), fl(MX, 2, NF), op=MIN)
    tt(fl(MN1, 0, NI), fl(Rmn, 0, NI), fl(Rmd, 0, NI), op=MIN)
    tt(fl(MX1, 0, NI), fl(Rmn, 0, NI), fl(Rmd, 0, NI), op=MAX)
    tt(fl(TF, 0, NI), fl(MX1, 0, NI), fl(Rmx, 0, NI), op=MIN)
    tt(fl(OUT, 2, 512), fl(MN1, 0, 510), fl(TF, 0, 510), op=MAX)

    gv = lambda T: T[:][0:NE, 0:511:510]
    bt = lambda tag: pm.tile([NE, 2], BF16, tag=f"e{tag}b", name=f"e{tag}b")
    BA, BB, BC = bt("BA"), bt("BB"), bt("BC")
    B1, B2, B3 = bt("B1"), bt("B2"), bt("B3")
    nc.vector.tensor_scalar_max(BA[:], gv(Pmn), 0.0)
    nc.vector.tensor_scalar_min(BC[:], gv(Pmx), 0.0)
    nc.vector.scalar_tensor_tensor(BB[:], gv(Qmx), 0.0, gv(Qmn), op0=MIN, op1=MAX)
    tt(B1[:], BA[:], BB[:], op=MIN)
    tt(B2[:], BA[:], BB[:], op=MAX)
    tt(B3[:], B2[:], BC[:], op=MIN)
    tt(OUT[:][0:NE, 1:513:511], B1[:], B3[:], op=MAX)


def build_program():
    nc = bacc.Bacc(
        "TRN2", target_bir_lowering=False, debug=False, num_devices=N_CORES
    )
    x_d = nc.dram_tensor("x", [B_PER, C, H, W], BF16, kind="ExternalInput").ap()
    o_d = nc.dram_tensor("out", [B_PER, C, H, W], BF16, kind="ExternalOutput").ap()
    xh = x_d.rearrange("b c h w -> h (b c) w")  # [512, 12, 512]
    oh = o_d.rearrange("b c h w -> h (b c) w")
    xi = x_d.rearrange("b c h w -> (b c) h w")  # [12, 512, 512]
    oi = o_d.rearrange("b c h w -> (b c) h w")

    with tile.TileContext(nc) as tc:
        with (
            tc.tile_pool(name="io", bufs=1) as pio,
            tc.tile_pool(name="mid", bufs=1) as pm,
        ):
            nb = 2 * (NIMG // GIMG)
            _block(nc, pio, pm, xh, oh, 0, 0, first=True)
            # edge loads are tiny; edge COMPUTE runs last, in the shadow of
            # the final block's output stores
            R0, R1 = _edge_loads(nc, pio, xi)
            for i in range(1, nb):
                _block(nc, pio, pm, xh, oh, i // 2, i % 2, last=(i == nb - 1))
            _edge_compute(nc, pio, pm, oi, R0, R1)
    nc.compile()
    return nc


def _get_program():
    global _PROGRAM
    if _PROGRAM is None:
        _PROGRAM = build_program()
    return _PROGRAM


def make_in_maps(x: np.ndarray):
    xb = np.ascontiguousarray(x).astype(ml_dtypes.bfloat16)
    return [{"x": xb[k * B_PER : (k + 1) * B_PER]} for k in range(N_CORES)]


def kernel(**inputs) -> np.ndarray:
    x = np.asarray(inputs["x"], dtype=np.float32)
    assert x.shape == (B, C, H, W), x.shape
    nc = _get_program()
    res = bass_utils.run_bass_kernel_spmd(
        nc, make_in_maps(x), core_ids=list(range(N_CORES))
    )
    out = np.concatenate(
        [np.asarray(res.results[k]["out"]) for k in range(N_CORES)], axis=0
    )
    return out.astype(np.float32)
